# revision 17
# baseline (speedup 1.0000x reference)
"""BertAttention (cross-attention, eval) on 8 Trainium2 NeuronCores.

Problem: B=4, SQ=SK=2048, HID=1024, NH=16, HD=64.
  q = hidden @ Wq + bq ; k = ctx @ Wk + bk ; v = ctx @ Wv + bv
  out = softmax(q k^T / 8) v        (per head), heads re-merged.

Sharding (no collectives): 8 cores = 4 batches x 2 head-groups.
Core c handles batch b = c//2 and heads hs..hs+8 where hs = (c%2)*8.

Math rearrangement (all exact):
  * softmax is shift-invariant per row, so k-bias terms cancel.  The
    surviving rank-1 term rT[k,h] = bq_h . K_h[k,:]/8 is folded in
    MULTIPLICATIVELY: exp(s + rT) = exp(s) * exp(rT), and exp(rT) is
    absorbed into the V rows (and the denominator ones-columns), so the
    exp() activation needs NO bias operand.  exp(rT) is computed on the
    host (it is a tiny [SK, NHC] matrix) and shipped as an input.
  * exp() is applied without max subtraction (scores ~ N(0,1), safe f32).
  * P@V is computed unnormalized with an er-column appended to V, so the
    PSUM accumulator row 64 holds the softmax denominator; reciprocal +
    broadcast multiply normalizes at the end, then + bv.

Layout: scores are built transposed (k on partitions, q free) so exp()
output PT feeds the P@V matmul directly as the moving operand.  Scores
for a head PAIR share one PSUM tile ([128, 1024] = h0 512q | h1 512q),
so one bias-free exp() covers both heads.  q is processed in blocks of
512 columns.

Pipeline: DMAs are column-tiled and issued in first-consumer order, so
the attention loop starts after a ~6MB prefix instead of the full 11MB.
The V projection and Q/K projection groups are emitted into the PE
slack of the ACT-bound attention steady state via an explicit
(pair, qblock, kc) work schedule.  Input tiles (except xT) are
double-buffered so consecutive in-NEFF reps overlap DMA with compute.
"""

import numpy as np
import ml_dtypes

import concourse.bass as bass
import concourse.mybir as mybir
import concourse.tile as tile
from concourse import bacc
from concourse.bass_utils import run_bass_kernel_spmd

P = 128
B, SQ, SK, HID, NH = 4, 2048, 2048, 1024, 16
HD = 64
N_CORES = 8
NHC = NH // 2          # heads per core = 8
DW = NHC * HD          # per-core projection width = 512
VW = NHC * (HD + 1)    # V block width per k-chunk (64 vals + er col per head)

_BF = ml_dtypes.bfloat16


def build_nc(sq=SQ, sk=SK, hid=HID, nhc=NHC, reps=1):
    hd = HD
    cc_n = hid // P          # contraction chunks (8)
    kc_n = sk // P           # key chunks (16)
    pairs = nhc // 2         # 4
    dw = nhc * hd            # 512
    vw = nhc * (hd + 1)      # 520
    qb_n = sq // 512         # q blocks (4)
    kg_n = sk // 512         # K-proj groups (4)
    qg_n = sq // 512         # Q-proj groups (4)

    bf = mybir.dt.bfloat16
    f32 = mybir.dt.float32
    Exp = mybir.ActivationFunctionType.Exp
    MULT = mybir.AluOpType.mult

    nc = bacc.Bacc("TRN2", target_bir_lowering=False, debug=False)

    xT = nc.dram_tensor("xT", [hid, sq], bf, kind="ExternalInput").ap()
    cT = nc.dram_tensor("cT", [hid, sk], bf, kind="ExternalInput").ap()
    wq = nc.dram_tensor("wq", [hid, dw], bf, kind="ExternalInput").ap()
    wk = nc.dram_tensor("wk", [hid, dw], bf, kind="ExternalInput").ap()
    wv = nc.dram_tensor("wv", [hid, dw], bf, kind="ExternalInput").ap()
    er = nc.dram_tensor("er", [P, kc_n * nhc], f32, kind="ExternalInput").ap()
    bv = nc.dram_tensor("bv", [dw], f32, kind="ExternalInput").ap()
    out = nc.dram_tensor("out", [dw, sq], f32, kind="ExternalOutput").ap()

    with tile.TileContext(nc) as tc:
        with (
            tc.tile_pool(name="in2", bufs=2) as ipool,     # rep-overlapped
            tc.tile_pool(name="in1", bufs=1) as xpool,     # frees mid-rep
            tc.tile_pool(name="qk", bufs=2) as qkpool,
            tc.tile_pool(name="pt", bufs=4) as ptpool,
            tc.tile_pool(name="work", bufs=2) as wpool,
            tc.tile_pool(name="psum", bufs=2, space="PSUM") as pspool,
        ):
            def alloc_tiles():
                t = {}
                t["xT_sb"] = xpool.tile([P, cc_n * sq], bf, name="xT_sb")
                t["cT_sb"] = ipool.tile([P, cc_n * sk], bf, name="cT_sb")
                t["wq_sb"] = xpool.tile([P, cc_n * dw], bf, name="wq_sb")
                t["wk_sb"] = xpool.tile([P, cc_n * dw], bf, name="wk_sb")
                t["wv_sb"] = xpool.tile([P, cc_n * dw], bf, name="wv_sb")
                t["v_sb"] = ipool.tile([P, kc_n * vw], bf, name="v_sb")
                t["er_sb"] = xpool.tile([P, kc_n * nhc], f32, name="er_sb")
                t["bv_sb"] = xpool.tile([hd, nhc], f32, name="bv_sb")
                t["qkt"] = {}
                t["proj_ps"] = {}
                return t

            def emit_dmas(t):
                # first-consumer order, column-tiled, one instruction per
                # (tensor, column-group) to keep the HWDGE queue short
                cT_d = t["cT_sb"].rearrange("p (c s) -> p c s", c=cc_n)
                cT_s = cT.rearrange("(c p) s -> p c s", p=P)
                xT_d = t["xT_sb"].rearrange("p (c s) -> p c s", c=cc_n)
                xT_s = xT.rearrange("(c p) s -> p c s", p=P)

                def col_group(dst, src, g):
                    nc.sync.dma_start(dst[:, :, g * 512:(g + 1) * 512],
                                      src[:, :, g * 512:(g + 1) * 512])

                nc.sync.dma_start(
                    t["wv_sb"].rearrange("p (c w) -> p c w", c=cc_n),
                    wv.rearrange("(c p) w -> p c w", p=P))
                col_group(cT_d, cT_s, 0)
                nc.sync.dma_start(t["er_sb"][:, :], er[:, :])
                nc.sync.dma_start(
                    t["wk_sb"].rearrange("p (c w) -> p c w", c=cc_n),
                    wk.rearrange("(c p) w -> p c w", p=P))
                col_group(cT_d, cT_s, 1)
                nc.sync.dma_start(
                    t["wq_sb"].rearrange("p (c w) -> p c w", c=cc_n),
                    wq.rearrange("(c p) w -> p c w", p=P))
                col_group(xT_d, xT_s, 0)
                col_group(cT_d, cT_s, 2)
                col_group(cT_d, cT_s, 3)
                for g in range(1, 4):
                    col_group(xT_d, xT_s, g)
                nc.sync.dma_start(t["bv_sb"][:, :],
                                  bv.rearrange("(h d) -> d h", d=hd))

            def emit_vproj(t, kc):
                pv_ps = pspool.tile([P, 512], f32, tag="pj", name="pv_ps")
                for cc in range(cc_n):
                    nc.tensor.matmul(
                        pv_ps[:, :],
                        lhsT=t["cT_sb"][:, cc * sk + kc * P: cc * sk + kc * P + P],
                        rhs=t["wv_sb"][:, cc * dw:(cc + 1) * dw],
                        start=(cc == 0), stop=(cc == cc_n - 1))
                base = kc * vw
                v_sb, er_sb = t["v_sb"], t["er_sb"]
                for h in range(nhc):
                    nc.vector.tensor_scalar_mul(
                        v_sb[:, base + h * (hd + 1): base + h * (hd + 1) + hd],
                        pv_ps[:, h * hd:(h + 1) * hd],
                        er_sb[:, kc * nhc + h: kc * nhc + h + 1])
                vdst = v_sb[:, base:base + vw].rearrange(
                    "p (h w) -> p h w", h=nhc)[:, :, hd:hd + 1]
                nc.vector.tensor_copy(
                    vdst, er_sb[:, kc * nhc:(kc + 1) * nhc].rearrange(
                        "p (h w) -> p h w", w=1))

            def get_qkt(t, pp):
                if pp not in t["qkt"]:
                    qt = qkpool.tile([P, sq], bf, tag="qt", name=f"qt{pp}")
                    kt = qkpool.tile([P, sk], bf, tag="kt", name=f"kt{pp}")
                    t["qkt"][pp] = (qt, kt)
                return t["qkt"][pp]

            def emit_proj_part(t, kind, pp, g, c0, c1):
                key = (kind, pp, g)
                if key not in t["proj_ps"]:
                    t["proj_ps"][key] = pspool.tile([P, 512], f32, tag="pj",
                                                    name=f"{kind}_ps")
                ps = t["proj_ps"][key]
                w_sb = t["wk_sb"] if kind == "k" else t["wq_sb"]
                src = t["cT_sb"] if kind == "k" else t["xT_sb"]
                for cc in range(c0, c1):
                    nc.tensor.matmul(
                        ps[:, :],
                        lhsT=w_sb[:, cc * dw + pp * P: cc * dw + pp * P + P],
                        rhs=src[:, cc * sk + g * 512: cc * sk + (g + 1) * 512],
                        start=(cc == 0), stop=(cc == cc_n - 1))
                if c1 == cc_n:
                    qt, kt = get_qkt(t, pp)
                    dst = kt if kind == "k" else qt
                    nc.vector.tensor_copy(dst[:, g * 512:(g + 1) * 512],
                                          ps[:, :])
                    del t["proj_ps"][key]

            def prologue_units(t, n_vproj):
                units = [(lambda k: lambda: emit_vproj(t, k))(kc)
                         for kc in range(n_vproj)]
                for c0 in range(0, cc_n, 2):
                    units.append((lambda c: lambda:
                                  emit_proj_part(t, "k", 0, 0, c, c + 2))(c0))
                for c0 in range(0, cc_n, 2):
                    units.append((lambda c: lambda:
                                  emit_proj_part(t, "q", 0, 0, c, c + 2))(c0))
                return units

            def emit_norm(state):
                p, qb = state["p"], state["qb"]
                bv_sb = state["t"]["bv_sb"]
                for hh, ctx_ps in ((0, state["ctx0"]), (1, state["ctx1"])):
                    h = 2 * p + hh
                    rec = wpool.tile([1, 512], f32, tag="rec", name="rec")
                    nc.vector.reciprocal(rec, ctx_ps[hd:hd + 1, :])
                    rec_bc = wpool.tile([hd, 512], f32, tag="recbc",
                                        name="rec_bc")
                    nc.gpsimd.partition_broadcast(rec_bc[:, :], rec[:, :])
                    o_sb = wpool.tile([hd, 512], f32, tag="osb", name="o_sb",
                                      bufs=6)
                    nc.vector.tensor_tensor(
                        o_sb[:, :], ctx_ps[0:hd, :], rec_bc[:, :], MULT)
                    nc.vector.tensor_scalar_add(o_sb[:, :], o_sb[:, :],
                                                bv_sb[:, h:h + 1])
                    nc.sync.dma_start(
                        out[p * P + hh * hd: p * P + (hh + 1) * hd,
                            qb * 512:(qb + 1) * 512],
                        o_sb[:, :])

            def emit_pv(state, kc, pt):
                p, v_sb = state["p"], state["t"]["v_sb"]
                h0, h1 = 2 * p, 2 * p + 1
                nc.tensor.matmul(
                    state["ctx0"][:, :],
                    lhsT=v_sb[:, kc * vw + h0 * (hd + 1):
                              kc * vw + (h0 + 1) * (hd + 1)],
                    rhs=pt[:, 0:512],
                    start=(kc == 0), stop=(kc == kc_n - 1))
                nc.tensor.matmul(
                    state["ctx1"][:, :],
                    lhsT=v_sb[:, kc * vw + h1 * (hd + 1):
                              kc * vw + (h1 + 1) * (hd + 1)],
                    rhs=pt[:, 512:1024],
                    start=(kc == 0), stop=(kc == kc_n - 1))

            def emit_body(t, next_t, chase_from, carry_over):
                """One rep's attention.  vproj(chase_from..) is scheduled into
                this rep's own pair-0 slots; next_t's input DMAs + prologue
                (carry_over units) are woven into pair 3."""
                sched = {}

                def add(p, qb, kc, fn):
                    sched.setdefault((p, qb, kc), []).append(fn)

                def add_split(p, qb, kc0, kind, pp, g):
                    for u in range(4):
                        add(p, qb, kc0 + u,
                            (lambda kd, ppp, gg, c0: lambda:
                             emit_proj_part(t, kd, ppp, gg, c0, c0 + 2))(
                                 kind, pp, g, 2 * u))

                for kc in range(chase_from, kc_n):
                    add(0, 0, kc - (chase_from - 1),
                        (lambda k: lambda: emit_vproj(t, k))(kc))
                add(0, 0, 2, lambda: emit_proj_part(t, "k", 0, 1, 0, cc_n))
                add(0, 0, 6, lambda: emit_proj_part(t, "k", 0, 2, 0, cc_n))
                add(0, 0, 10, lambda: emit_proj_part(t, "k", 0, 3, 0, cc_n))
                add(0, 0, 14, lambda: emit_proj_part(t, "q", 0, 1, 0, cc_n))
                add_split(0, 1, 0, "q", 0, 2)
                add_split(0, 1, 4, "q", 0, 3)
                for p in range(pairs - 1):
                    for g in range(kg_n):
                        add_split(p, 2, 4 * g, "k", p + 1, g)
                    for g in range(qg_n):
                        add_split(p, 3, 4 * g, "q", p + 1, g)
                # weave the next rep's prologue into pair 3
                for u, fn in enumerate(carry_over):
                    qb, kc = 2 + u // 8, 2 * (u % 8) + 1
                    add(3, qb, kc, fn)

                prev = None
                state = None
                for i in range(pairs * qb_n * kc_n):
                    p, r = divmod(i, qb_n * kc_n)
                    qb, kc = divmod(r, kc_n)
                    if next_t is not None and p == 3 and qb == 0 and kc == 0:
                        emit_dmas(next_t)
                    qt_sb, kt_sb = get_qkt(t, p)
                    if kc == 0:
                        t["qkt"].pop(p - 1, None)
                        ctx0 = pspool.tile([hd + 1, 512], f32, tag="ctx",
                                           name="ctx0")
                        ctx1 = pspool.tile([hd + 1, 512], f32, tag="ctx",
                                           name="ctx1")
                        state = {"p": p, "qb": qb, "ctx0": ctx0, "ctx1": ctx1,
                                 "t": t}
                    qs = qb * 512
                    st = pspool.tile([P, 1024], f32, tag="st", name="st")
                    nc.tensor.matmul(
                        st[:, 0:512],
                        lhsT=kt_sb[0:64, kc * P:(kc + 1) * P],
                        rhs=qt_sb[0:64, qs:qs + 512],
                        start=True, stop=True, tile_position=(0, 0))
                    nc.tensor.matmul(
                        st[:, 512:1024],
                        lhsT=kt_sb[64:128, kc * P:(kc + 1) * P],
                        rhs=qt_sb[64:128, qs:qs + 512],
                        start=True, stop=True, tile_position=(64, 0))
                    pt = ptpool.tile([P, 1024], bf, tag="pt", name="pt")
                    nc.scalar.activation(pt, st, Exp)
                    for fn in sched.pop((p, qb, kc), ()):
                        fn()
                    if prev is not None:
                        pstate = prev[0]
                        emit_pv(*prev)
                        if prev[1] == kc_n - 1:
                            emit_norm(pstate)
                    prev = (state, kc, pt)
                emit_pv(*prev)
                emit_norm(prev[0])
                assert not sched, f"unscheduled work: {list(sched)}"

            t = alloc_tiles()
            emit_dmas(t)
            for fn in prologue_units(t, 4):
                fn()
            chase = 4
            for r in range(reps):
                next_t = alloc_tiles() if r + 1 < reps else None
                carry = prologue_units(next_t, 8) if next_t else []
                emit_body(t, next_t, chase, carry)
                t = next_t
                chase = 8

    nc.compile()
    return nc


_NC_CACHE = {}


def _get_nc():
    if "nc" not in _NC_CACHE:
        _NC_CACHE["nc"] = build_nc()
    return _NC_CACHE["nc"]


def _prep_core_inputs(hidden_states, context, Wq, bq, Wk, bk, Wv, bv):
    """Host-side shard + layout prep. Returns list of 8 in_maps."""
    scale = 1.0 / np.sqrt(HD)
    xT_b, cT_b = [], []
    for b in range(B):
        xT_b.append(np.ascontiguousarray(hidden_states[b].T).astype(_BF))
        cT_b.append(np.ascontiguousarray(context[b].T).astype(_BF))
    in_maps = []
    for c in range(N_CORES):
        b = c // 2
        hs = (c % 2) * NHC
        cols = slice(hs * HD, (hs + NHC) * HD)
        wq_c = (Wq[:, cols] * scale).astype(_BF)
        wk_c = Wk[:, cols].astype(_BF)
        wv_c = Wv[:, cols].astype(_BF)
        # er[k, h] = exp(rT) with rT = (C @ (Wk_h @ bq_h)) * scale
        wkr = np.empty((HID, NHC), np.float32)
        for h in range(NHC):
            hcols = slice((hs + h) * HD, (hs + h + 1) * HD)
            wkr[:, h] = (Wk[:, hcols] @ bq[hcols]) * scale
        rT = np.asarray(context[b], np.float32) @ wkr        # [SK, NHC]
        er_c = np.exp(rT).reshape(SK // P, P, NHC).transpose(1, 0, 2)
        er_c = np.ascontiguousarray(er_c.reshape(P, -1), np.float32)
        in_maps.append({
            "xT": xT_b[b],
            "cT": cT_b[b],
            "wq": np.ascontiguousarray(wq_c),
            "wk": np.ascontiguousarray(wk_c),
            "wv": np.ascontiguousarray(wv_c),
            "er": er_c,
            "bv": np.ascontiguousarray(bv[cols]).astype(np.float32),
        })
    return in_maps


def kernel(hidden_states, context, Wq, bq, Wk, bk, Wv, bv):
    hidden_states = np.asarray(hidden_states, dtype=np.float32)
    context = np.asarray(context, dtype=np.float32)
    Wq = np.asarray(Wq, dtype=np.float32)
    bq = np.asarray(bq, dtype=np.float32)
    Wk = np.asarray(Wk, dtype=np.float32)
    bk = np.asarray(bk, dtype=np.float32)
    Wv = np.asarray(Wv, dtype=np.float32)
    bv = np.asarray(bv, dtype=np.float32)

    nc = _get_nc()
    in_maps = _prep_core_inputs(hidden_states, context, Wq, bq, Wk, bk, Wv, bv)
    res = run_bass_kernel_spmd(nc, in_maps, list(range(N_CORES)))
    full = np.empty((B, SQ, NH * HD), np.float32)
    for c in range(N_CORES):
        b = c // 2
        hs = (c % 2) * NHC
        cols = slice(hs * HD, (hs + NHC) * HD)
        full[b, :, cols] = res.results[c]["out"].T
    return full
